# revision 1
# baseline (speedup 1.0000x reference)
"""Cross-attention kernel for Trainium2, 8-core SPMD.

Problem (all fp32):
  x [2, 2048, 1024]; wq/wk/wv/w_proj [1024, 1024]; b_proj [1024]
  q = x[:, :1024] @ wq.T   (16 heads x 64)
  k, v = x @ wk.T, x @ wv.T
  out = softmax(q k^T / 8) v  -> proj + bias  -> [2, 1024, 1024]

Sharding: 8 cores = 2 (batch) x 4 (head-groups of 4 heads). Each core
computes its batch's QKV for its 4 heads, full attention for those heads,
and a partial projection (its 256 contraction rows of w_proj). Host sums
the 4 partials per batch and adds the bias (standard tensor-parallel
unshard).

Per-core layout ("T convention"): activations are kept feature-on-partition
(xT [c, n]); q/k are produced transposed (qT/kT [d, n]), v natural [n, d]
with an appended ones-column so the attn@v matmul also emits the softmax
denominator for free. The softmax max-subtraction is skipped (scores are
provably < ~10 for this problem, exp stays in fp32 range).

Schedule: inputs stream in chunk-interleaved across both DMA queue
families while q/k(pair0) and half the v-projection consume each x chunk
as it lands; scores(0) then runs with the rest of stage A interleaved as
PE filler (phased so every exp's SBUF slot is freed by earlier-emitted
work - the PE queue is strict FIFO and slot waits can otherwise
deadlock); attnv(h-1) interleaves per-j with scores(h) so the ACT
engine's exp stream (~73us floor) stays saturated; the projection tail
alternates evacuation engines and output DMA queues.
"""

import os
import numpy as np

import concourse.bacc as bacc
import concourse.bass as bass
import concourse.tile as tile
import concourse.mybir as mybir
from concourse.bass_utils import run_bass_kernel_spmd

F32 = mybir.dt.float32
# float32r: same fp32 bits, single-pass PE matmul (4x faster than fp32's
# two half-speed passes) at 11-bit-mantissa internal precision.
MM_DT = {
    "f32": mybir.dt.float32,
    "f32r": mybir.dt.float32r,
}[os.environ.get("KERNEL_MM_DT", "f32r")]

C = 1024          # model dim
N = 2048          # kv tokens
NQ = 1024         # query tokens
HPC = 4           # heads per core
D = 64            # head dim
DH = HPC * D      # per-core slice of C (256)
SCALE = D ** -0.5
P = 128

_CACHE: dict = {}


def _build():
    nc = bacc.Bacc("TRN2", target_bir_lowering=False, debug=False, num_devices=8)

    xT = nc.dram_tensor("xT", [C, N], MM_DT, kind="ExternalInput").ap()
    wqT = nc.dram_tensor("wqT", [C, DH], MM_DT, kind="ExternalInput").ap()
    wkT = nc.dram_tensor("wkT", [C, DH], MM_DT, kind="ExternalInput").ap()
    wvT = nc.dram_tensor("wvT", [C, DH], MM_DT, kind="ExternalInput").ap()
    wpT = nc.dram_tensor("wpT", [DH, C], MM_DT, kind="ExternalInput").ap()
    out = nc.dram_tensor("out", [NQ, C], F32, kind="ExternalOutput").ap()

    with tile.TileContext(nc) as tc, \
            nc.allow_low_precision(reason="fp32r matmul pipeline (fp32 bits, 11-bit mantissa in PE)"):
        _emit(tc, xT, wqT, wkT, wvT, wpT, out)

    nc.compile()
    return nc


def _emit(tc, xT, wqT, wkT, wvT, wpT, out):
    nc = tc.nc
    mm = nc.tensor.matmul
    Exp = mybir.ActivationFunctionType.Exp

    from contextlib import ExitStack

    with ExitStack() as ctx:
        # One shared slot class for every [128, 2048]-f32-sized tile: the 8
        # xT chunks + 3 QKV weights live through stage A, then those slots
        # recycle as exp(scores) tiles during attention.
        big = ctx.enter_context(tc.tile_pool(name="big", bufs=15))
        singles = ctx.enter_context(tc.tile_pool(name="singles", bufs=1))
        rcp = ctx.enter_context(tc.tile_pool(name="rcp", bufs=1))
        bcp = ctx.enter_context(tc.tile_pool(name="bcp", bufs=1))
        outp = ctx.enter_context(tc.tile_pool(name="outp", bufs=4))
        ps_big = ctx.enter_context(tc.tile_pool(name="ps_big", bufs=3, space="PSUM"))
        ps_sm = ctx.enter_context(tc.tile_pool(name="ps_sm", bufs=2, space="PSUM"))

        # ---- loads (per-chunk weight DMAs so the first matmul starts after
        # ~256KB of traffic instead of ~2MB; in first-use order)
        def load_w(name, dram):
            t = big.tile([P, 8, DH], MM_DT, name=name, tag="big")
            src = dram.rearrange("(a p) d -> p a d", p=P)
            for ci in range(8):
                nc.sync.dma_start(out=t[:, ci, :], in_=src[:, ci, :])
            return t

        wq_src = wqT.rearrange("(a p) d -> p a d", p=P)
        wk_src = wkT.rearrange("(a p) d -> p a d", p=P)
        wq_sb = big.tile([P, 8, DH], MM_DT, name="wq_sb", tag="big")
        wk_sb = big.tile([P, 8, DH], MM_DT, name="wk_sb", tag="big")
        xt = []
        for ci in range(8):
            t = big.tile([P, N], MM_DT, name=f"xt{ci}", tag="big")
            xt.append(t)
        # Two DMA queue families run concurrently: HWDGE (nc.sync) carries
        # wq + even x chunks, SWDGE (nc.gpsimd) carries wk + odd x chunks,
        # interleaved so chunk ci's inputs land just before its matmuls.
        wv_sb = big.tile([P, 8, DH], MM_DT, name="wv_sb", tag="big")
        wv_src = wvT.rearrange("(a p) d -> p a d", p=P)
        nc.sync.dma_start(out=wq_sb[:, 0, :], in_=wq_src[:, 0, :])
        nc.gpsimd.dma_start(out=wk_sb[:, 0, :], in_=wk_src[:, 0, :])
        nc.sync.dma_start(out=xt[0], in_=xT[0:P, :])
        nc.gpsimd.dma_start(out=xt[1], in_=xT[P:2 * P, :])
        for ci in range(1, 4):
            nc.sync.dma_start(out=wq_sb[:, ci, :], in_=wq_src[:, ci, :])
            nc.gpsimd.dma_start(out=wk_sb[:, ci, :], in_=wk_src[:, ci, :])
        nc.sync.dma_start(out=wv_sb[:, 0, :], in_=wv_src[:, 0, :])
        nc.gpsimd.dma_start(out=wv_sb[:, 1, :], in_=wv_src[:, 1, :])
        nc.sync.dma_start(out=xt[2], in_=xT[2 * P:3 * P, :])
        nc.gpsimd.dma_start(out=xt[3], in_=xT[3 * P:4 * P, :])
        for ci in range(2, 4):
            eng = nc.sync if ci % 2 == 0 else nc.gpsimd
            eng.dma_start(out=wv_sb[:, ci, :], in_=wv_src[:, ci, :])
        for ci in range(4, 6):
            nc.sync.dma_start(out=wq_sb[:, ci, :], in_=wq_src[:, ci, :])
            nc.gpsimd.dma_start(out=wk_sb[:, ci, :], in_=wk_src[:, ci, :])
        nc.sync.dma_start(out=xt[4], in_=xT[4 * P:5 * P, :])
        nc.gpsimd.dma_start(out=xt[5], in_=xT[5 * P:6 * P, :])
        for ci in range(6, 8):
            nc.sync.dma_start(out=wq_sb[:, ci, :], in_=wq_src[:, ci, :])
            nc.gpsimd.dma_start(out=wk_sb[:, ci, :], in_=wk_src[:, ci, :])
        for ci in range(4, 8):
            eng = nc.sync if ci % 2 == 0 else nc.gpsimd
            eng.dma_start(out=wv_sb[:, ci, :], in_=wv_src[:, ci, :])
        nc.sync.dma_start(out=xt[6], in_=xT[6 * P:7 * P, :])
        nc.gpsimd.dma_start(out=xt[7], in_=xT[7 * P:8 * P, :])


        ones_sb = singles.tile([P, D], MM_DT, name="ones", tag="ones")
        nc.vector.memset(ones_sb.bitcast(F32), 1.0)

        # Pre-trigger the ~2.7us exp table load while DMAs stream.
        dm = singles.tile([1, 1], MM_DT, name="dm", tag="dm")
        nc.scalar.activation(out=dm, in_=ones_sb[0:1, 0:1], func=Exp, scale=1.0)

        # ---- stage A: q/k/v projection emitters --------------------------
        qt = [singles.tile([P, NQ], MM_DT, name=f"qt{p}", tag=f"qt{p}") for p in range(2)]
        kt = [singles.tile([P, N], MM_DT, name=f"kt{p}", tag=f"kt{p}") for p in range(2)]
        v_sb = []
        for j in range(16):
            t = singles.tile([P, HPC, D + 1], MM_DT, name=f"v{j}", tag=f"v{j}")
            v_sb.append(t)

        def q_proj_gen(pair):
            ps = ps_big.tile([P, 1024], F32, name=f"ps_q{pair}", tag="psb")
            for ci in range(8):
                lw = wq_sb[:, ci, pair * P:(pair + 1) * P]
                for nh in range(2):
                    mm(ps[:, nh * 512:(nh + 1) * 512], lw,
                       xt[ci][:, nh * 512:(nh + 1) * 512],
                       start=(ci == 0), stop=(ci == 7), skip_group_check=True)
                yield
            nc.vector.tensor_copy(qt[pair], ps)
            yield

        def k_proj_gen(pair, half):
            ps = ps_big.tile([P, 1024], F32, name=f"ps_k{pair}_{half}", tag="psb")
            for ci in range(8):
                lw = wk_sb[:, ci, pair * P:(pair + 1) * P]
                for nh in range(2):
                    nk0 = half * 1024 + nh * 512
                    mm(ps[:, nh * 512:(nh + 1) * 512], lw,
                       xt[ci][:, nk0:nk0 + 512],
                       start=(ci == 0), stop=(ci == 7), skip_group_check=True)
                yield
            nc.vector.tensor_copy(kt[pair][:, half * 1024:(half + 1) * 1024], ps)
            yield

        def v_group_gen(j):
            # v pass 2 (ci 4..7), accumulated onto pass 1's partial in SBUF
            ps = ps_sm.tile([P, 512], F32, name=f"ps_v2_{j}", tag="pss")
            for ci in range(4, 8):
                mm(ps[:, 0:DH], xt[ci][:, j * P:(j + 1) * P],
                   wv_sb[:, ci, :],
                   start=(ci == 4), stop=(ci == 7), skip_group_check=True)
                yield
            nc.vector.tensor_add(
                v_sb[j][:, :, 0:D], v_sb[j][:, :, 0:D],
                ps[:, 0:DH].rearrange("p (h d) -> p h d", h=HPC))
            yield

        # ---- attention helpers -------------------------------------------
        out_h = [singles.tile([D, NQ], MM_DT, name=f"oh{h}", tag=f"oh{h}")
                 for h in range(HPC)]

        def alloc_ets(h):
            return [big.tile([P, 2, NQ], MM_DT, name=f"et{h}_{k}", tag="big")
                    for k in range(8)]

        def scores_j(h, ets, j):
            pair, po = h // 2, 64 * (h % 2)
            ps = ps_big.tile([P, 1024], F32, name=f"ps_s{h}_{j}", tag="psb")
            lw = kt[pair][po:po + 64, j * P:(j + 1) * P]
            for nh in range(2):
                mm(ps[:, nh * 512:(nh + 1) * 512], lw,
                   qt[pair][po:po + 64, nh * 512:(nh + 1) * 512],
                   start=True, stop=True)
            nc.scalar.activation(out=ets[j // 2][:, j % 2, :], in_=ps,
                                 func=Exp, scale=SCALE)

        def attnv_j(h, ets, ps_o, j):
            lw = v_sb[j][:, h, :]               # [128, 65] (col 64 = ones)
            for nh in range(2):
                mm(ps_o[0:D + 1, nh * 512:(nh + 1) * 512], lw,
                   ets[j // 2][:, j % 2, nh * 512:(nh + 1) * 512],
                   start=(j == 0), stop=(j == 15), skip_group_check=True)

        def norm(h, ps_o):
            # rows 0..63 = unnormalized out^T, row 64 = sum(exp) denominator
            rc = rcp.tile([D + 1, NQ], MM_DT, name=f"rc{h}", tag="rc")
            nc.vector.reciprocal(rc[D:D + 1, :], ps_o[D:D + 1, :])
            # broadcast 1/denom across partitions via ones-outer-product
            bc = bcp.tile([D, NQ], MM_DT, name=f"bc{h}", tag="bc")
            for nh in range(2):
                pb = ps_sm.tile([P, 512], F32, name=f"ps_b{h}_{nh}", tag="pss")
                mm(pb[0:D, :], ones_sb[D:D + 1, 0:D],
                   rc[D:D + 1, nh * 512:(nh + 1) * 512],
                   start=True, stop=True)
                nc.vector.tensor_copy(bc[:, nh * 512:(nh + 1) * 512], pb[0:D, :])
            nc.vector.tensor_mul(out_h[h], ps_o[0:D, :], bc)

        # ---- A1: q/k for head-pair 0, ci-outer so each arriving xT chunk
        # is consumed immediately (3 psum groups accumulate in parallel) ---
        ps_qa = ps_big.tile([P, 1024], F32, name="ps_q0", tag="psb")
        ps_ka = [ps_big.tile([P, 1024], F32, name=f"ps_k0_{half}", tag="psb")
                 for half in range(2)]

        def a1_part(cis):
            for ci in cis:
                lw = wq_sb[:, ci, 0:P]
                for nh in range(2):
                    mm(ps_qa[:, nh * 512:(nh + 1) * 512], lw,
                       xt[ci][:, nh * 512:(nh + 1) * 512],
                       start=(ci == 0), stop=(ci == 7), skip_group_check=True)
                lw = wk_sb[:, ci, 0:P]
                for half in range(2):
                    for nh in range(2):
                        nk0 = half * 1024 + nh * 512
                        mm(ps_ka[half][:, nh * 512:(nh + 1) * 512], lw,
                           xt[ci][:, nk0:nk0 + 512],
                           start=(ci == 0), stop=(ci == 7), skip_group_check=True)

        # v passes 1a/1b (ci 0,1 then 2,3) are placed exactly at the two
        # input-arrival waits (xt2/3 and xt4/5); pass 2 finishes in B0.
        a1_part(range(2))
        for j in range(16):
            ps = ps_sm.tile([P, 512], F32, name=f"ps_v1a_{j}", tag="pss")
            for ci in range(2):
                mm(ps[:, 0:DH], xt[ci][:, j * P:(j + 1) * P],
                   wv_sb[:, ci, :],
                   start=(ci == 0), stop=(ci == 1), skip_group_check=True)
            nc.vector.tensor_copy(
                v_sb[j][:, :, 0:D],
                ps[:, 0:DH].rearrange("p (h d) -> p h d", h=HPC))
            nc.gpsimd.memset(v_sb[j][:, :, D:D + 1].bitcast(F32), 1.0)
        a1_part(range(2, 4))
        for j in range(16):
            ps = ps_sm.tile([P, 512], F32, name=f"ps_v1b_{j}", tag="pss")
            for ci in range(2, 4):
                mm(ps[:, 0:DH], xt[ci][:, j * P:(j + 1) * P],
                   wv_sb[:, ci, :],
                   start=(ci == 2), stop=(ci == 3), skip_group_check=True)
            nc.vector.tensor_add(
                v_sb[j][:, :, 0:D], v_sb[j][:, :, 0:D],
                ps[:, 0:DH].rearrange("p (h d) -> p h d", h=HPC))
        a1_part(range(4, 8))
        nc.vector.tensor_copy(qt[0], ps_qa)
        for half in range(2):
            nc.vector.tensor_copy(kt[0][:, half * 1024:(half + 1) * 1024],
                                  ps_ka[half])

        # ---- B0: scores(0) with the rest of stage A as PE filler ---------
        # PE is strict FIFO, so each scores_j may only be emitted after the
        # filler whose completion frees the SBUF slot its exp needs:
        # 4 slots are free at B0 start (exps j0..7), +1 after q(1) (j8,9),
        # +1 after k(1,1) (j10,11), and the rest only after v releases the
        # xT chunks (j12..15 come last).
        from itertools import chain

        def pull(gen, k):
            for _ in range(k):
                if next(gen, None) is None:
                    return False
            return True

        ets_prev = alloc_ets(0)
        f1 = q_proj_gen(1)                                   # 17 units
        for j in range(6):
            scores_j(0, ets_prev, j)
            pull(f1, 3)
        for _ in f1:
            pass
        f2 = chain(k_proj_gen(1, 0), k_proj_gen(1, 1))       # 34 units
        for j in range(6, 10):
            scores_j(0, ets_prev, j)
            pull(f2, 9)
        for _ in f2:
            pass
        f3 = chain(*(v_group_gen(j) for j in range(16)))     # 80 units
        for j in range(10, 12):
            scores_j(0, ets_prev, j)
            pull(f3, 12)
        for _ in f3:
            pass
        for j in range(12, 16):
            scores_j(0, ets_prev, j)

        # ---- pipelined attention: attnv(h-1) interleaved with scores(h) --
        ps_o_prev = ps_big.tile([P, 1024], F32, name="ps_o0", tag="psb")
        for h in range(1, HPC):
            ets_h = alloc_ets(h)
            ps_o_h = None
            for j in range(16):
                scores_j(h, ets_h, j)
                attnv_j(h - 1, ets_prev, ps_o_prev, j)
            norm(h - 1, ps_o_prev)
            ets_prev = ets_h
            ps_o_prev = ps_big.tile([P, 1024], F32, name=f"ps_o{h}", tag="psb")
        wp_h = []
        for h in range(HPC):
            t = big.tile([D, C], MM_DT, name=f"wp{h}", tag="big")
            nc.sync.dma_start(out=t, in_=wpT[h * D:(h + 1) * D, :])
            wp_h.append(t)
        for j in range(16):
            attnv_j(HPC - 1, ets_prev, ps_o_prev, j)
        norm(HPC - 1, ps_o_prev)

        # ---- partial projection ------------------------------------------
        for m in range(8):
            ps = ps_big.tile([P, 1024], F32, name=f"ps_f{m}", tag="psb")
            for h in range(HPC):
                lw = out_h[h][:, m * P:(m + 1) * P]   # [64, 128]
                for nh in range(2):
                    mm(ps[:, nh * 512:(nh + 1) * 512], lw,
                       wp_h[h][:, nh * 512:(nh + 1) * 512],
                       start=(h == 0), stop=(h == HPC - 1), skip_group_check=True)
            fin = outp.tile([P, 1024], F32, name=f"fin{m}", tag="fin")
            nc.scalar.copy(fin[:, 0:512], ps[:, 0:512])
            nc.vector.tensor_copy(fin[:, 512:1024], ps[:, 512:1024])
            nc.sync.dma_start(out=out[m * P:(m + 1) * P, 0:512],
                              in_=fin[:, 0:512])
            nc.gpsimd.dma_start(out=out[m * P:(m + 1) * P, 512:1024],
                                in_=fin[:, 512:1024])


def _get_nc():
    if "nc" not in _CACHE:
        _CACHE["nc"] = _build()
    return _CACHE["nc"]


def kernel(x, wq, wk, wv, w_proj, b_proj):
    x = np.asarray(x, dtype=np.float32)
    wq = np.asarray(wq, dtype=np.float32)
    wk = np.asarray(wk, dtype=np.float32)
    wv = np.asarray(wv, dtype=np.float32)
    w_proj = np.asarray(w_proj, dtype=np.float32)
    b_proj = np.asarray(b_proj, dtype=np.float32)

    nc = _get_nc()
    in_maps = []
    for core in range(8):
        b, g = divmod(core, 4)
        sl = slice(g * DH, (g + 1) * DH)
        in_maps.append({
            "xT": np.ascontiguousarray(x[b].T),
            "wqT": np.ascontiguousarray(wq[sl, :].T),
            "wkT": np.ascontiguousarray(wk[sl, :].T),
            "wvT": np.ascontiguousarray(wv[sl, :].T),
            "wpT": np.ascontiguousarray(w_proj[:, sl].T),
        })

    res = run_bass_kernel_spmd(nc, in_maps, core_ids=list(range(8)),
                               trace=bool(int(os.environ.get("KERNEL_TRACE", "0"))))
    _CACHE["last_results"] = res
    outs = [res.results[c]["out"] for c in range(8)]
    full = np.stack([outs[0] + outs[1] + outs[2] + outs[3],
                     outs[4] + outs[5] + outs[6] + outs[7]])
    full += b_proj[None, None, :]
    return full.astype(np.float32)



# revision 32
# speedup vs baseline: 1.3191x; 1.3191x over previous
"""Cross-attention kernel for Trainium2, 8-core SPMD (v3: bf16 + transposed attnv).

Problem (all fp32):
  x [2, 2048, 1024]; wq/wk/wv/w_proj [1024, 1024]; b_proj [1024]
  q = x[:, :1024] @ wq.T   (16 heads x 64)
  k, v = x @ wk.T, x @ wv.T
  out = softmax(q k^T / 8) v  -> proj + bias  -> [2, 1024, 1024]

Sharding: 8 cores = 2 (batch) x 4 (head-groups of 4 heads = 2 pairs of 2).
Each core emits TWO bf16 partials (one per head-pair); host upcasts, sums
the 16 partials per batch and adds the bias.

Design (matmul cost = out-cols x 0.4167ns x cpr; bf16 cpr=1 at any width,
fp32r cpr=4 below 256 cols):
  - x/weights stream in as bf16 (half DMA bytes); q/k kept fp32r so scores
    stay high precision; exp output, v, attn, proj all bf16.
  - attnv is transposed: stationary = exp tile [128kv, 128q], moving =
    v [128kv, 65] -> psum [q-block, 65].  8320 cols/head vs 16384, and the
    ones-column denominator lands per-PARTITION, so normalization is a
    cheap DVE tensor_scalar multiply (no PE broadcast matmuls).
  - normalized attn for a head-pair is packed [128q, 128dd], transposed
    (pair0: DMA-xbar mid-kernel; pair1: PE transpose in the tail where
    PSUM is free) and projected with a full-128 contraction.
  - PSUM (8 banks): scores [128, 1024] x2 (4) + attnv 2 x [128, 4, 65]
    (2) + one [128, 1024] rotating "seq" slot (2) for k1a/k1b/v_j/proj0.
    q pair1 runs inside stage A (its own psum there is the seq slot's
    first user).  Tail projection alternates the sc and seq tags for
    2-deep pipelining.
  - The exp stream (64 x [128, 1024], ~1.07us each) is the pacing engine;
    PE in-loop work is levelled across the 4 head loops so ACT never
    starves: h0 carries v j0..11, h1 carries k1 + v j12..15 + attnv(h0),
    h2 carries attnv(h1), h3 carries attnv(h2) + attnv(h3) + proj0.
"""

import os
import numpy as np
import ml_dtypes

import concourse.bacc as bacc
import concourse.bass as bass
import concourse.tile as tile
import concourse.mybir as mybir
from concourse.bass_utils import run_bass_kernel_spmd
from concourse.masks import make_identity

F32 = mybir.dt.float32
F32R = mybir.dt.float32r
BF16 = mybir.dt.bfloat16

C = 1024          # model dim
N = 2048          # kv tokens
NQ = 1024         # query tokens
HPC = 4           # heads per core
D = 64            # head dim
DH = HPC * D      # per-core slice of C (256)
SCALE = D ** -0.5
P = 128

_CACHE: dict = {}
_BF = ml_dtypes.bfloat16


def _build():
    nc = bacc.Bacc("TRN2", target_bir_lowering=False, debug=False, num_devices=8)

    xT = nc.dram_tensor("xT", [C, N], BF16, kind="ExternalInput").ap()
    # wqk{p} = hstack(wq[pair p slice].T, wk[pair p slice].T)  [C, 256]
    wqk0 = nc.dram_tensor("wqk0", [C, 2 * P], BF16, kind="ExternalInput").ap()
    wqk1 = nc.dram_tensor("wqk1", [C, 2 * P], BF16, kind="ExternalInput").ap()
    wvT = nc.dram_tensor("wvT", [C, DH], BF16, kind="ExternalInput").ap()
    wpT = nc.dram_tensor("wpT", [DH, C], BF16, kind="ExternalInput").ap()
    outA = nc.dram_tensor("outA", [NQ, C], BF16, kind="ExternalOutput").ap()
    outB = nc.dram_tensor("outB", [NQ, C], BF16, kind="ExternalOutput").ap()

    with tile.TileContext(nc) as tc, \
            nc.allow_low_precision(reason="bf16 pipeline within 2e-2 tolerance"):
        _emit(tc, xT, wqk0, wqk1, wvT, wpT, outA, outB)

    nc.compile()
    return nc


def _emit(tc, xT, wqk0, wqk1, wvT, wpT, outA, outB):
    nc = tc.nc
    mm = nc.tensor.matmul
    Exp = mybir.ActivationFunctionType.Exp
    Copy = mybir.ActivationFunctionType.Copy

    from contextlib import ExitStack
    from itertools import chain

    with ExitStack() as ctx:
        singles = ctx.enter_context(tc.tile_pool(name="singles", bufs=1))
        ets_pool = ctx.enter_context(tc.tile_pool(name="ets", bufs=32))
        finp = ctx.enter_context(tc.tile_pool(name="finp", bufs=8))
        ps_sc = ctx.enter_context(tc.tile_pool(name="ps_sc", bufs=2, space="PSUM"))
        ps_av = ctx.enter_context(tc.tile_pool(name="ps_av", bufs=2, space="PSUM"))
        ps_sq = ctx.enter_context(tc.tile_pool(name="ps_sq", bufs=2, space="PSUM"))

        # ---------------- input DMAs (one ordered SP/HWDGE stream) --------
        # Per chunk: wqk0_ci, wqk1_ci, x_ci  (stage A consumes q0/k0/q1 per
        # chunk as it lands); then wv, wp (needed from ~h0/h3).
        xt = [singles.tile([P, N], BF16, name=f"xt{ci}", tag=f"xt{ci}")
              for ci in range(8)]
        wqk0_sb = singles.tile([P, 8, 2 * P], BF16, name="wqk0_sb", tag="wqk0")
        wqk1_sb = singles.tile([P, 8, 2 * P], BF16, name="wqk1_sb", tag="wqk1")
        wv_sb = singles.tile([P, 8, DH], BF16, name="wv_sb", tag="wv")
        wp_sb = [singles.tile([P, C], BF16, name=f"wp{p}", tag=f"wp{p}")
                 for p in range(2)]

        wqk0_src = wqk0.rearrange("(a p) d -> p a d", p=P)
        wqk1_src = wqk1.rearrange("(a p) d -> p a d", p=P)
        wv_src = wvT.rearrange("(a p) d -> p a d", p=P)

        for ci in range(8):
            nc.sync.dma_start(out=wqk0_sb[:, ci, :], in_=wqk0_src[:, ci, :])
            nc.sync.dma_start(out=xt[ci], in_=xT[ci * P:(ci + 1) * P, :])
        for ci in range(8):
            nc.sync.dma_start(out=wqk1_sb[:, ci, :], in_=wqk1_src[:, ci, :])
            nc.sync.dma_start(out=wv_sb[:, ci, :], in_=wv_src[:, ci, :])
        for p in range(2):
            nc.sync.dma_start(out=wp_sb[p], in_=wpT[p * P:(p + 1) * P, :])

        # ---------------- small consts ----------------
        identity = singles.tile([P, P], BF16, name="identity", tag="ident")
        make_identity(nc, identity)

        # Pre-trigger the exp table load while DMAs stream.
        dmt = singles.tile([1, 1], BF16, name="dmt", tag="dmt")
        nc.scalar.activation(out=dmt, in_=identity[0:1, 0:1], func=Exp, scale=1.0)

        # ---------------- persistent SBUF ----------------
        qt = [singles.tile([P, NQ], F32R, name=f"qt{p}", tag=f"qt{p}")
              for p in range(2)]
        kt = [singles.tile([P, N], F32R, name=f"kt{p}", tag=f"kt{p}")
              for p in range(2)]
        v_sb = singles.tile([P, 16, HPC, D + 1], BF16, name="v_sb", tag="v_sb")
        nc.gpsimd.memset(v_sb[:, :, :, D:D + 1], 1.0)

        attn_pack = [singles.tile([P, 8, P], BF16, name=f"apk{p}", tag=f"apk{p}")
                     for p in range(2)]
        attn_T = [singles.tile([P, 8, P], BF16, name=f"atT{p}", tag=f"atT{p}")
                  for p in range(2)]
        rcp = singles.tile([P, HPC, 8], F32, name="rcp", tag="rcp")

        # ---------------- stage A: q pair0 + k pair0 + q1 quarters --------
        # 8 mm per chunk vs ~1.6us chunk arrival: roughly DMA-paced.  q1 is
        # split into four [128, 256] quarter-psums so two of them fit the
        # (otherwise idle) 1-bank attnv slots during stage A; the other two
        # run as the first h0 fillers.
        ps_q0a = ps_sq.tile([P, 512], F32, name="ps_q0a", tag="sq")
        ps_q0b = ps_sq.tile([P, 512], F32, name="ps_q0b", tag="sq")
        ps_k0a = ps_sc.tile([P, NQ], F32, name="ps_k0a", tag="sc")
        ps_k0b = ps_sc.tile([P, NQ], F32, name="ps_k0b", tag="sc")
        ps_q1a = ps_av.tile([P, 256], F32, name="ps_q1a", tag="av")
        ps_q1b = ps_av.tile([P, 256], F32, name="ps_q1b", tag="av")
        for ci in range(8):
            lw_q0 = wqk0_sb[:, ci, 0:P]
            lw_k0 = wqk0_sb[:, ci, P:2 * P]
            lw_q1 = wqk1_sb[:, ci, 0:P]
            st = dict(start=(ci == 0), stop=(ci == 7), skip_group_check=True)
            for nh in range(2):
                sl = slice(nh * 512, (nh + 1) * 512)
                mm((ps_q0a if nh == 0 else ps_q0b), lw_q0, xt[ci][:, sl], **st)
                mm(ps_k0a[:, sl], lw_k0, xt[ci][:, sl], **st)
                mm(ps_k0b[:, sl], lw_k0, xt[ci][:, 1024 + nh * 512:1024 + (nh + 1) * 512], **st)
            mm(ps_q1a, lw_q1, xt[ci][:, 0:256], **st)
            mm(ps_q1b, lw_q1, xt[ci][:, 256:512], **st)
        # evacs split across DVE+ACT+Pool (all idle pre-stream) to shorten
        # the serial path to the first scores matmul
        nc.vector.tensor_copy(qt[0][:, 0:512], ps_q0a)
        nc.scalar.copy(qt[0][:, 512:1024], ps_q0b)
        nc.vector.tensor_copy(kt[0][:, 0:512], ps_k0a[:, 0:512])
        nc.scalar.copy(kt[0][:, 512:1024], ps_k0a[:, 512:1024])
        nc.vector.tensor_copy(kt[0][:, 1024:1536], ps_k0b[:, 0:512])
        nc.scalar.copy(kt[0][:, 1536:2048], ps_k0b[:, 512:1024])
        nc.vector.tensor_copy(qt[1][:, 0:256], ps_q1a)
        nc.vector.tensor_copy(qt[1][:, 256:512], ps_q1b)

        # ---------------- fillers ----------------
        MM = 0.427  # us per 512-col matmul at full clock (cost bookkeeping)

        def q1cd_gen():
            # q1 quarters C/D through the freed attnv slots (xt resident)
            ps_c = ps_av.tile([P, 256], F32, name="ps_q1c", tag="av")
            for ci in range(8):
                mm(ps_c, wqk1_sb[:, ci, 0:P], xt[ci][:, 512:768],
                   start=(ci == 0), stop=(ci == 7), skip_group_check=True)
                yield 107
            ps_d = ps_av.tile([P, 256], F32, name="ps_q1d", tag="av")
            for ci in range(8):
                mm(ps_d, wqk1_sb[:, ci, 0:P], xt[ci][:, 768:1024],
                   start=(ci == 0), stop=(ci == 7), skip_group_check=True)
                yield 107
            nc.vector.tensor_copy(qt[1][:, 512:768], ps_c)
            nc.vector.tensor_copy(qt[1][:, 768:1024], ps_d)
            yield 0

        def k1_gen(quarter):
            ps = ps_sq.tile([P, 512], F32, name=f"ps_k1{quarter}", tag="sq")
            nk0 = quarter * 512
            for ci in range(8):
                lw = wqk1_sb[:, ci, P:2 * P]
                mm(ps, lw, xt[ci][:, nk0:nk0 + 512],
                   start=(ci == 0), stop=(ci == 7), skip_group_check=True)
                yield 213
            nc.vector.tensor_copy(kt[1][:, nk0:nk0 + 512], ps)
            yield 0

        def v_gen(t):
            # 2 kv-blocks (j = 2t, 2t+1) share one psum slot; one wide evac
            ps = ps_sq.tile([P, 2, DH], F32, name=f"ps_v{t}", tag="sq")
            for ci in range(8):
                for jj in range(2):
                    j = 2 * t + jj
                    # one start per psum BANK: jj=1's first matmul relies on
                    # the pending-zero left by jj=0's start
                    mm(ps[:, jj, :], xt[ci][:, j * P:(j + 1) * P],
                       wv_sb[:, ci, :],
                       start=(ci == 0 and jj == 0),
                       stop=(ci == 7 and jj == 1), skip_group_check=True)
                yield 214
            nc.vector.tensor_copy(
                v_sb[:, 2 * t:2 * t + 2, :, 0:D],
                ps.rearrange("p j (h d) -> p j h d", h=HPC))
            yield 0

        def proj0_gen(m):
            # two independent half-column units -> 2-wide through the sq slots
            for nh in range(2):
                ps = ps_sq.tile([P, 512], F32, name=f"ps_pj0_{m}_{nh}", tag="sq")
                mm(ps, attn_T[0][:, m, :],
                   wp_sb[0][:, nh * 512:(nh + 1) * 512],
                   start=True, stop=True, skip_group_check=True)
                yield 213
                fin = finp.tile([P, 512], BF16, name=f"fin0_{m}_{nh}", tag="fin")
                nc.vector.tensor_copy(fin, ps)
                nc.sync.dma_start(
                    out=outA[m * P:(m + 1) * P, nh * 512:(nh + 1) * 512],
                    in_=fin)
                yield 0

        # ---------------- attention pieces ----------------
        av_tiles = {}

        def alloc_av(h):
            av_tiles[h] = [ps_av.tile([P, 4, D + 1], F32, name=f"av{h}_{s}",
                                      tag="av") for s in range(2)]

        ets = {}

        def scores_j(h, j):
            pair, po = h // 2, D * (h % 2)
            ps = ps_sc.tile([P, NQ], F32, name=f"ps_s{h}_{j}", tag="sc")
            lw = kt[pair][po:po + D, j * P:(j + 1) * P]
            for nh in range(2):
                mm(ps[:, nh * 512:(nh + 1) * 512], lw,
                   qt[pair][po:po + D, nh * 512:(nh + 1) * 512],
                   start=True, stop=True, skip_group_check=True)
            et = ets_pool.tile([P, NQ], BF16, name=f"et{h}_{j}", tag="ets")
            nc.scalar.activation(out=et, in_=ps, func=Exp, scale=SCALE)
            ets[(h, j)] = et

        def attnv_j(h, j):
            et = ets[(h, j)]
            for qb in range(8):
                av = av_tiles[h][qb // 4]
                mm(av[:, qb % 4, :],
                   et[:, qb * P:(qb + 1) * P],
                   v_sb[:, j, h, :],
                   start=(j == 0 and qb % 4 == 0),
                   stop=(j == 15 and qb % 4 == 3),
                   skip_group_check=True)

        def norm_half(h, part, tail):
            # tail=False: DVE + Pool (ACT is mid-exp-stream); tail=True:
            # DVE + ACT (lower latency, stream over)
            pair, half = h // 2, h % 2
            av = av_tiles[h][part]
            nc.vector.reciprocal(rcp[:, h, part * 4:(part + 1) * 4], av[:, :, D])
            for i in range(4):
                qb = part * 4 + i
                dst = attn_pack[pair][:, qb, half * D:(half + 1) * D]
                if tail and i % 2 == 1:
                    nc.scalar.activation(out=dst, in_=av[:, i, 0:D], func=Copy,
                                         scale=rcp[:, h, qb:qb + 1])
                else:
                    nc.vector.tensor_scalar_mul(dst, av[:, i, 0:D],
                                                rcp[:, h, qb:qb + 1])

        def pull(gen, budget):
            # cost-aware: drain up to ~budget ns of emitted matmul work
            acc = 0
            while acc < budget:
                c = next(gen, None)
                if c is None:
                    return False
                acc += c
            return True

        # ---------------- head loops (ACT exp stream is the pacer) --------
        # Each head's 16 exps give ~17.1us of ACT; scores are ~6.8us of PE,
        # leaving ~640ns/iter of PE filler budget.
        # h0: v pairs 0..4 (j0..9) + q1 quarters C/D
        f = chain(*(v_gen(t) for t in range(5)), q1cd_gen())
        for j in range(16):
            scores_j(0, j)
            pull(f, 640)
        for _ in f:
            pass

        # h1: k1a + k1b first (unblocks h2 scores), then v pairs 5, 6
        f = chain(*(k1_gen(qu) for qu in range(4)), *(v_gen(t) for t in (5, 6)))
        for j in range(16):
            scores_j(1, j)
            pull(f, 640)
        for _ in f:
            pass

        # h2: v pair 7 early + attnv(h0) iters 0..4, norm(h0)@5,
        #     attnv(h1) 6..11, norm(h1)@12, pair0 transpose @12
        A0 = [(0, 3), (3, 6), (6, 9), (9, 11), (11, 14), (14, 16)]
        A1 = [(0, 3), (3, 5), (5, 8), (8, 10), (10, 13), (13, 16)]
        alloc_av(0)
        fv = chain(v_gen(7))
        fp = chain(*(proj0_gen(m) for m in range(8)))

        def tp0(qb):
            tp = ps_av.tile([P, P], BF16, name=f"tp0_{qb}", tag="av")
            nc.tensor.transpose(tp, attn_pack[0][:, qb, :], identity)
            nc.vector.tensor_copy(attn_T[0][:, qb, :], tp)

        for j in range(16):
            scores_j(2, j)
            if j < 6:
                for jj in range(*A0[j]):
                    attnv_j(0, jj)
            elif j == 6:
                norm_half(0, 0, False)
                norm_half(0, 1, False)
                alloc_av(1)
            elif j < 13:
                for jj in range(*A1[j - 7]):
                    attnv_j(1, jj)
            elif j == 13:
                norm_half(1, 0, False)
                norm_half(1, 1, False)
            elif j >= 14:
                for qb in (2 * (j - 14), 2 * (j - 14) + 1):
                    tp0(qb)
            if j < 4:
                pull(fv, 430)

        # h3: attnv(h2) iters 0..7, norm(h2)@8, attnv(h3) j0..14 iters 8..15,
        #     rest of proj0 spread over all iters
        alloc_av(2)
        for j in range(16):
            scores_j(3, j)
            if j < 2:
                tp0(4 + 2 * j)
                tp0(5 + 2 * j)
            if j < 8:
                attnv_j(2, 2 * j)
                attnv_j(2, 2 * j + 1)
            else:
                if j == 8:
                    norm_half(2, 0, False)
                    norm_half(2, 1, False)
                    alloc_av(3)
                for jj in range((j - 8) * 15 // 8, (j - 7) * 15 // 8):
                    attnv_j(3, jj)
            if j >= 1:
                pull(fp, 470)
        for _ in fp:
            pass

        # ---------------- tail ----------------
        attnv_j(3, 15)

        # Per-qb chain: normalize -> PE transpose -> evac -> project -> fin
        # -> DMA, with DVE/ACT/Pool round-robin so no single evac engine
        # serializes the drain.  proj psums 2-deep via the sc tag.
        av3 = av_tiles[3]
        nc.vector.reciprocal(rcp[:, 3, 0:4], av3[0][:, :, D])
        nc.vector.reciprocal(rcp[:, 3, 4:8], av3[1][:, :, D])

        def mul3(qb):
            dst = attn_pack[1][:, qb, D:2 * D]
            src_ = av3[qb // 4][:, qb % 4, 0:D]
            if qb % 2 == 0:
                nc.vector.tensor_scalar_mul(dst, src_, rcp[:, 3, qb:qb + 1])
            else:
                nc.scalar.activation(out=dst, in_=src_, func=Copy,
                                     scale=rcp[:, 3, qb:qb + 1])

        def tp1(qb):
            tp = ps_sq.tile([P, P], BF16, name=f"tp{qb}", tag="sq")
            nc.tensor.transpose(tp, attn_pack[1][:, qb, :], identity)
            if qb % 2 == 0:
                nc.vector.tensor_copy(attn_T[1][:, qb, :], tp)
            else:
                nc.scalar.copy(attn_T[1][:, qb, :], tp)

        # pair1 projection at quarter-column granularity: 32 independent
        # [128, 256] psum units spread across all three free pools (6 slots
        # in flight), single-engine fin per quarter (round-robin), one DMA
        # per m-block.
        qslots = [(ps_av, "av"), (ps_sc, "sc"), (ps_sq, "sq")]

        def proj1(m):
            fin = finp.tile([P, C], BF16, name=f"fin1_{m}", tag="fin")
            for qo in range(4):
                k = 4 * m + qo
                pool, tag = qslots[k % 3]
                ps = pool.tile([P, 256], F32, name=f"pj1_{m}_{qo}", tag=tag)
                mm(ps, attn_T[1][:, m, :],
                   wp_sb[1][:, qo * 256:(qo + 1) * 256],
                   start=True, stop=True, skip_group_check=True)
                dst = fin[:, qo * 256:(qo + 1) * 256]
                if k % 2 == 0:
                    nc.scalar.copy(dst, ps)
                else:
                    nc.vector.tensor_copy(dst, ps)
            nc.sync.dma_start(out=outB[m * P:(m + 1) * P, :], in_=fin)

        mul3(0)
        tp1(0)
        mul3(1)
        tp1(1)
        for qb in range(2, 8):
            mul3(qb)
            tp1(qb)
            proj1(qb - 2)
        proj1(6)
        proj1(7)


def _get_nc():
    if "nc" not in _CACHE:
        _CACHE["nc"] = _build()
    return _CACHE["nc"]


def kernel(x, wq, wk, wv, w_proj, b_proj):
    x = np.asarray(x, dtype=np.float32)
    wq = np.asarray(wq, dtype=np.float32)
    wk = np.asarray(wk, dtype=np.float32)
    wv = np.asarray(wv, dtype=np.float32)
    w_proj = np.asarray(w_proj, dtype=np.float32)
    b_proj = np.asarray(b_proj, dtype=np.float32)

    nc = _get_nc()
    in_maps = []
    for core in range(8):
        b, g = divmod(core, 4)
        s0 = g * DH
        p0 = slice(s0, s0 + P)            # pair0 rows (heads 4g, 4g+1)
        p1 = slice(s0 + P, s0 + 2 * P)    # pair1 rows
        sl = slice(s0, s0 + DH)
        in_maps.append({
            "xT": np.ascontiguousarray(x[b].T).astype(_BF),
            "wqk0": np.ascontiguousarray(
                np.hstack([wq[p0, :].T, wk[p0, :].T])).astype(_BF),
            "wqk1": np.ascontiguousarray(
                np.hstack([wq[p1, :].T, wk[p1, :].T])).astype(_BF),
            "wvT": np.ascontiguousarray(wv[sl, :].T).astype(_BF),
            "wpT": np.ascontiguousarray(w_proj[:, sl].T).astype(_BF),
        })

    res = run_bass_kernel_spmd(nc, in_maps, core_ids=list(range(8)),
                               trace=bool(int(os.environ.get("KERNEL_TRACE", "0"))))
    _CACHE["last_results"] = res
    acc = [np.zeros((NQ, C), np.float32) for _ in range(2)]
    for core in range(8):
        b = core // 4
        acc[b] += res.results[core]["outA"].astype(np.float32)
        acc[b] += res.results[core]["outB"].astype(np.float32)
    full = np.stack(acc)
    full += b_proj[None, None, :]
    return full.astype(np.float32)


# revision 33
# speedup vs baseline: 1.3247x; 1.0042x over previous
"""Cross-attention kernel for Trainium2, 8-core SPMD (v3: bf16 + transposed attnv).

Problem (all fp32):
  x [2, 2048, 1024]; wq/wk/wv/w_proj [1024, 1024]; b_proj [1024]
  q = x[:, :1024] @ wq.T   (16 heads x 64)
  k, v = x @ wk.T, x @ wv.T
  out = softmax(q k^T / 8) v  -> proj + bias  -> [2, 1024, 1024]

Sharding: 8 cores = 2 (batch) x 4 (head-groups of 4 heads = 2 pairs of 2).
Each core emits TWO bf16 partials (one per head-pair); host upcasts, sums
the 16 partials per batch and adds the bias.

Design (matmul cost = out-cols x 0.4167ns x cpr; bf16 cpr=1 at any width,
fp32r cpr=4 below 256 cols):
  - x/weights stream in as bf16 (half DMA bytes); q/k kept fp32r so scores
    stay high precision; exp output, v, attn, proj all bf16.
  - attnv is transposed: stationary = exp tile [128kv, 128q], moving =
    v [128kv, 65] -> psum [q-block, 65].  8320 cols/head vs 16384, and the
    ones-column denominator lands per-PARTITION, so normalization is a
    cheap DVE tensor_scalar multiply (no PE broadcast matmuls).
  - normalized attn for a head-pair is packed [128q, 128dd], transposed
    (pair0: DMA-xbar mid-kernel; pair1: PE transpose in the tail where
    PSUM is free) and projected with a full-128 contraction.
  - PSUM (8 banks): scores [128, 1024] x2 (4) + attnv 2 x [128, 4, 65]
    (2) + one [128, 1024] rotating "seq" slot (2) for k1a/k1b/v_j/proj0.
    q pair1 runs inside stage A (its own psum there is the seq slot's
    first user).  Tail projection alternates the sc and seq tags for
    2-deep pipelining.
  - The exp stream (64 x [128, 1024], ~1.07us each) is the pacing engine;
    PE in-loop work is levelled across the 4 head loops so ACT never
    starves: h0 carries v j0..11, h1 carries k1 + v j12..15 + attnv(h0),
    h2 carries attnv(h1), h3 carries attnv(h2) + attnv(h3) + proj0.
"""

import os
import numpy as np
import ml_dtypes

import concourse.bacc as bacc
import concourse.bass as bass
import concourse.tile as tile
import concourse.mybir as mybir
from concourse.bass_utils import run_bass_kernel_spmd
from concourse.masks import make_identity

F32 = mybir.dt.float32
F32R = mybir.dt.float32r
BF16 = mybir.dt.bfloat16

C = 1024          # model dim
N = 2048          # kv tokens
NQ = 1024         # query tokens
HPC = 4           # heads per core
D = 64            # head dim
DH = HPC * D      # per-core slice of C (256)
SCALE = D ** -0.5
P = 128

_CACHE: dict = {}
_BF = ml_dtypes.bfloat16


def _build():
    nc = bacc.Bacc("TRN2", target_bir_lowering=False, debug=False, num_devices=8)

    xT = nc.dram_tensor("xT", [C, N], BF16, kind="ExternalInput").ap()
    # wqk{p} = hstack(wq[pair p slice].T, wk[pair p slice].T)  [C, 256]
    wqk0 = nc.dram_tensor("wqk0", [C, 2 * P], BF16, kind="ExternalInput").ap()
    wqk1 = nc.dram_tensor("wqk1", [C, 2 * P], BF16, kind="ExternalInput").ap()
    wvT = nc.dram_tensor("wvT", [C, DH], BF16, kind="ExternalInput").ap()
    wpT = nc.dram_tensor("wpT", [DH, C], BF16, kind="ExternalInput").ap()
    outA = nc.dram_tensor("outA", [NQ, C], BF16, kind="ExternalOutput").ap()
    outB = nc.dram_tensor("outB", [NQ, C], BF16, kind="ExternalOutput").ap()

    with tile.TileContext(nc) as tc, \
            nc.allow_low_precision(reason="bf16 pipeline within 2e-2 tolerance"):
        _emit(tc, xT, wqk0, wqk1, wvT, wpT, outA, outB)

    nc.compile()
    return nc


def _emit(tc, xT, wqk0, wqk1, wvT, wpT, outA, outB):
    nc = tc.nc
    mm = nc.tensor.matmul
    Exp = mybir.ActivationFunctionType.Exp
    Copy = mybir.ActivationFunctionType.Copy

    from contextlib import ExitStack
    from itertools import chain

    with ExitStack() as ctx:
        singles = ctx.enter_context(tc.tile_pool(name="singles", bufs=1))
        ets_pool = ctx.enter_context(tc.tile_pool(name="ets", bufs=32))
        finp = ctx.enter_context(tc.tile_pool(name="finp", bufs=8))
        ps_sc = ctx.enter_context(tc.tile_pool(name="ps_sc", bufs=2, space="PSUM"))
        ps_av = ctx.enter_context(tc.tile_pool(name="ps_av", bufs=2, space="PSUM"))
        ps_sq = ctx.enter_context(tc.tile_pool(name="ps_sq", bufs=2, space="PSUM"))

        # ---------------- input DMAs (one ordered SP/HWDGE stream) --------
        # Per chunk: wqk0_ci, wqk1_ci, x_ci  (stage A consumes q0/k0/q1 per
        # chunk as it lands); then wv, wp (needed from ~h0/h3).
        xt = [singles.tile([P, N], BF16, name=f"xt{ci}", tag=f"xt{ci}")
              for ci in range(8)]
        wqk0_sb = singles.tile([P, 8, 2 * P], BF16, name="wqk0_sb", tag="wqk0")
        wqk1_sb = singles.tile([P, 8, 2 * P], BF16, name="wqk1_sb", tag="wqk1")
        wv_sb = singles.tile([P, 8, DH], BF16, name="wv_sb", tag="wv")
        wp_sb = [singles.tile([P, C], BF16, name=f"wp{p}", tag=f"wp{p}")
                 for p in range(2)]

        wqk0_src = wqk0.rearrange("(a p) d -> p a d", p=P)
        wqk1_src = wqk1.rearrange("(a p) d -> p a d", p=P)
        wv_src = wvT.rearrange("(a p) d -> p a d", p=P)

        for ci in range(8):
            nc.sync.dma_start(out=wqk0_sb[:, ci, :], in_=wqk0_src[:, ci, :])
            if ci >= 6:
                nc.sync.dma_start(out=xt[ci][:, 0:1024],
                                  in_=xT[ci * P:(ci + 1) * P, 0:1024])
                nc.sync.dma_start(out=xt[ci][:, 1024:2048],
                                  in_=xT[ci * P:(ci + 1) * P, 1024:2048])
            else:
                nc.sync.dma_start(out=xt[ci], in_=xT[ci * P:(ci + 1) * P, :])
        for ci in range(8):
            nc.sync.dma_start(out=wqk1_sb[:, ci, :], in_=wqk1_src[:, ci, :])
            nc.sync.dma_start(out=wv_sb[:, ci, :], in_=wv_src[:, ci, :])
        for p in range(2):
            nc.sync.dma_start(out=wp_sb[p], in_=wpT[p * P:(p + 1) * P, :])

        # ---------------- small consts ----------------
        identity = singles.tile([P, P], BF16, name="identity", tag="ident")
        make_identity(nc, identity)

        # Pre-trigger the exp table load while DMAs stream.
        dmt = singles.tile([1, 1], BF16, name="dmt", tag="dmt")
        nc.scalar.activation(out=dmt, in_=identity[0:1, 0:1], func=Exp, scale=1.0)

        # ---------------- persistent SBUF ----------------
        qt = [singles.tile([P, NQ], F32R, name=f"qt{p}", tag=f"qt{p}")
              for p in range(2)]
        kt = [singles.tile([P, N], F32R, name=f"kt{p}", tag=f"kt{p}")
              for p in range(2)]
        v_sb = singles.tile([P, 16, HPC, D + 1], BF16, name="v_sb", tag="v_sb")
        nc.gpsimd.memset(v_sb[:, :, :, D:D + 1], 1.0)

        attn_pack = [singles.tile([P, 8, P], BF16, name=f"apk{p}", tag=f"apk{p}")
                     for p in range(2)]
        attn_T = [singles.tile([P, 8, P], BF16, name=f"atT{p}", tag=f"atT{p}")
                  for p in range(2)]
        rcp = singles.tile([P, HPC, 8], F32, name="rcp", tag="rcp")

        # ---------------- stage A: q pair0 + k pair0 + q1 quarters --------
        # 8 mm per chunk vs ~1.6us chunk arrival: roughly DMA-paced.  q1 is
        # split into four [128, 256] quarter-psums so two of them fit the
        # (otherwise idle) 1-bank attnv slots during stage A; the other two
        # run as the first h0 fillers.
        ps_q0a = ps_sq.tile([P, 512], F32, name="ps_q0a", tag="sq")
        ps_q0b = ps_sq.tile([P, 512], F32, name="ps_q0b", tag="sq")
        ps_k0a = ps_sc.tile([P, NQ], F32, name="ps_k0a", tag="sc")
        ps_k0b = ps_sc.tile([P, NQ], F32, name="ps_k0b", tag="sc")
        ps_q1a = ps_av.tile([P, 256], F32, name="ps_q1a", tag="av")
        ps_q1b = ps_av.tile([P, 256], F32, name="ps_q1b", tag="av")
        for ci in range(8):
            lw_q0 = wqk0_sb[:, ci, 0:P]
            lw_k0 = wqk0_sb[:, ci, P:2 * P]
            lw_q1 = wqk1_sb[:, ci, 0:P]
            st = dict(start=(ci == 0), stop=(ci == 7), skip_group_check=True)
            mm(ps_q0a, lw_q0, xt[ci][:, 0:512], **st)
            mm(ps_k0a[:, 0:512], lw_k0, xt[ci][:, 0:512], **st)
            mm(ps_q0b, lw_q0, xt[ci][:, 512:1024], **st)
            mm(ps_k0a[:, 512:1024], lw_k0, xt[ci][:, 512:1024], **st)
            mm(ps_k0b[:, 0:512], lw_k0, xt[ci][:, 1024:1536], **st)
            mm(ps_k0b[:, 512:1024], lw_k0, xt[ci][:, 1536:2048], **st)
            mm(ps_q1a, lw_q1, xt[ci][:, 0:256], **st)
            mm(ps_q1b, lw_q1, xt[ci][:, 256:512], **st)
        # evacs split across DVE+ACT+Pool (all idle pre-stream) to shorten
        # the serial path to the first scores matmul
        nc.vector.tensor_copy(qt[0][:, 0:512], ps_q0a)
        nc.scalar.copy(qt[0][:, 512:1024], ps_q0b)
        nc.vector.tensor_copy(kt[0][:, 0:512], ps_k0a[:, 0:512])
        nc.scalar.copy(kt[0][:, 512:1024], ps_k0a[:, 512:1024])
        nc.vector.tensor_copy(kt[0][:, 1024:1536], ps_k0b[:, 0:512])
        nc.scalar.copy(kt[0][:, 1536:2048], ps_k0b[:, 512:1024])
        nc.vector.tensor_copy(qt[1][:, 0:256], ps_q1a)
        nc.vector.tensor_copy(qt[1][:, 256:512], ps_q1b)

        # ---------------- fillers ----------------
        MM = 0.427  # us per 512-col matmul at full clock (cost bookkeeping)

        def q1cd_gen():
            # q1 quarters C/D through the freed attnv slots (xt resident)
            ps_c = ps_av.tile([P, 256], F32, name="ps_q1c", tag="av")
            for ci in range(8):
                mm(ps_c, wqk1_sb[:, ci, 0:P], xt[ci][:, 512:768],
                   start=(ci == 0), stop=(ci == 7), skip_group_check=True)
                yield 107
            ps_d = ps_av.tile([P, 256], F32, name="ps_q1d", tag="av")
            for ci in range(8):
                mm(ps_d, wqk1_sb[:, ci, 0:P], xt[ci][:, 768:1024],
                   start=(ci == 0), stop=(ci == 7), skip_group_check=True)
                yield 107
            nc.vector.tensor_copy(qt[1][:, 512:768], ps_c)
            nc.vector.tensor_copy(qt[1][:, 768:1024], ps_d)
            yield 0

        def k1_gen(quarter):
            ps = ps_sq.tile([P, 512], F32, name=f"ps_k1{quarter}", tag="sq")
            nk0 = quarter * 512
            for ci in range(8):
                lw = wqk1_sb[:, ci, P:2 * P]
                mm(ps, lw, xt[ci][:, nk0:nk0 + 512],
                   start=(ci == 0), stop=(ci == 7), skip_group_check=True)
                yield 213
            nc.vector.tensor_copy(kt[1][:, nk0:nk0 + 512], ps)
            yield 0

        def v_gen(t):
            # 2 kv-blocks (j = 2t, 2t+1) share one psum slot; one wide evac
            ps = ps_sq.tile([P, 2, DH], F32, name=f"ps_v{t}", tag="sq")
            for ci in range(8):
                for jj in range(2):
                    j = 2 * t + jj
                    # one start per psum BANK: jj=1's first matmul relies on
                    # the pending-zero left by jj=0's start
                    mm(ps[:, jj, :], xt[ci][:, j * P:(j + 1) * P],
                       wv_sb[:, ci, :],
                       start=(ci == 0 and jj == 0),
                       stop=(ci == 7 and jj == 1), skip_group_check=True)
                yield 214
            nc.vector.tensor_copy(
                v_sb[:, 2 * t:2 * t + 2, :, 0:D],
                ps.rearrange("p j (h d) -> p j h d", h=HPC))
            yield 0

        def proj0_gen(m):
            # two independent half-column units -> 2-wide through the sq slots
            for nh in range(2):
                ps = ps_sq.tile([P, 512], F32, name=f"ps_pj0_{m}_{nh}", tag="sq")
                mm(ps, attn_T[0][:, m, :],
                   wp_sb[0][:, nh * 512:(nh + 1) * 512],
                   start=True, stop=True, skip_group_check=True)
                yield 213
                fin = finp.tile([P, 512], BF16, name=f"fin0_{m}_{nh}", tag="fin")
                nc.vector.tensor_copy(fin, ps)
                nc.sync.dma_start(
                    out=outA[m * P:(m + 1) * P, nh * 512:(nh + 1) * 512],
                    in_=fin)
                yield 0

        # ---------------- attention pieces ----------------
        av_tiles = {}

        def alloc_av(h):
            av_tiles[h] = [ps_av.tile([P, 4, D + 1], F32, name=f"av{h}_{s}",
                                      tag="av") for s in range(2)]

        ets = {}

        def scores_j(h, j):
            pair, po = h // 2, D * (h % 2)
            ps = ps_sc.tile([P, NQ], F32, name=f"ps_s{h}_{j}", tag="sc")
            lw = kt[pair][po:po + D, j * P:(j + 1) * P]
            for nh in range(2):
                mm(ps[:, nh * 512:(nh + 1) * 512], lw,
                   qt[pair][po:po + D, nh * 512:(nh + 1) * 512],
                   start=True, stop=True, skip_group_check=True)
            et = ets_pool.tile([P, NQ], BF16, name=f"et{h}_{j}", tag="ets")
            nc.scalar.activation(out=et, in_=ps, func=Exp, scale=SCALE)
            ets[(h, j)] = et

        def attnv_j(h, j):
            et = ets[(h, j)]
            for qb in range(8):
                av = av_tiles[h][qb // 4]
                mm(av[:, qb % 4, :],
                   et[:, qb * P:(qb + 1) * P],
                   v_sb[:, j, h, :],
                   start=(j == 0 and qb % 4 == 0),
                   stop=(j == 15 and qb % 4 == 3),
                   skip_group_check=True)

        def norm_half(h, part, tail):
            # tail=False: DVE + Pool (ACT is mid-exp-stream); tail=True:
            # DVE + ACT (lower latency, stream over)
            pair, half = h // 2, h % 2
            av = av_tiles[h][part]
            nc.vector.reciprocal(rcp[:, h, part * 4:(part + 1) * 4], av[:, :, D])
            for i in range(4):
                qb = part * 4 + i
                dst = attn_pack[pair][:, qb, half * D:(half + 1) * D]
                if tail and i % 2 == 1:
                    nc.scalar.activation(out=dst, in_=av[:, i, 0:D], func=Copy,
                                         scale=rcp[:, h, qb:qb + 1])
                else:
                    nc.vector.tensor_scalar_mul(dst, av[:, i, 0:D],
                                                rcp[:, h, qb:qb + 1])

        def pull(gen, budget):
            # cost-aware: drain up to ~budget ns of emitted matmul work
            acc = 0
            while acc < budget:
                c = next(gen, None)
                if c is None:
                    return False
                acc += c
            return True

        # ---------------- head loops (ACT exp stream is the pacer) --------
        # Each head's 16 exps give ~17.1us of ACT; scores are ~6.8us of PE,
        # leaving ~640ns/iter of PE filler budget.
        # h0: v pairs 0..4 (j0..9) + q1 quarters C/D
        f = chain(*(v_gen(t) for t in range(5)), q1cd_gen())
        for j in range(16):
            scores_j(0, j)
            pull(f, 640)
        for _ in f:
            pass

        # h1: k1a + k1b first (unblocks h2 scores), then v pairs 5, 6
        f = chain(*(k1_gen(qu) for qu in range(4)), *(v_gen(t) for t in (5, 6)))
        for j in range(16):
            scores_j(1, j)
            pull(f, 600)
        for _ in f:
            pass

        # h2: v pair 7 early + attnv(h0) iters 0..4, norm(h0)@5,
        #     attnv(h1) 6..11, norm(h1)@12, pair0 transpose @12
        A0 = [(0, 3), (3, 6), (6, 9), (9, 11), (11, 14), (14, 16)]
        A1 = [(0, 3), (3, 5), (5, 8), (8, 10), (10, 13), (13, 16)]
        alloc_av(0)
        fv = chain(v_gen(7))
        fp = chain(*(proj0_gen(m) for m in range(8)))

        def tp0(qb):
            tp = ps_av.tile([P, P], BF16, name=f"tp0_{qb}", tag="av")
            nc.tensor.transpose(tp, attn_pack[0][:, qb, :], identity)
            nc.vector.tensor_copy(attn_T[0][:, qb, :], tp)

        for j in range(16):
            scores_j(2, j)
            if j < 6:
                for jj in range(*A0[j]):
                    attnv_j(0, jj)
            elif j == 6:
                norm_half(0, 0, False)
                norm_half(0, 1, False)
                alloc_av(1)
            elif j < 13:
                for jj in range(*A1[j - 7]):
                    attnv_j(1, jj)
            elif j == 13:
                norm_half(1, 0, False)
                norm_half(1, 1, False)
            elif j >= 14:
                for qb in (2 * (j - 14), 2 * (j - 14) + 1):
                    tp0(qb)
            if j < 4:
                pull(fv, 430)

        # h3: attnv(h2) iters 0..7, norm(h2)@8, attnv(h3) j0..14 iters 8..15,
        #     rest of proj0 spread over all iters
        alloc_av(2)
        for j in range(16):
            scores_j(3, j)
            if j < 2:
                tp0(4 + 2 * j)
                tp0(5 + 2 * j)
            if j < 8:
                attnv_j(2, 2 * j)
                attnv_j(2, 2 * j + 1)
            else:
                if j == 8:
                    norm_half(2, 0, False)
                    norm_half(2, 1, False)
                    alloc_av(3)
                for jj in range((j - 8) * 15 // 8, (j - 7) * 15 // 8):
                    attnv_j(3, jj)
            if 1 <= j <= 6:
                pull(fp, 520)
            elif j >= 10:
                pull(fp, 570)
        for _ in fp:
            pass

        # ---------------- tail ----------------
        attnv_j(3, 15)

        # Per-qb chain: normalize -> PE transpose -> evac -> project -> fin
        # -> DMA, with DVE/ACT/Pool round-robin so no single evac engine
        # serializes the drain.  proj psums 2-deep via the sc tag.
        av3 = av_tiles[3]
        nc.vector.reciprocal(rcp[:, 3, 0:4], av3[0][:, :, D])
        nc.vector.reciprocal(rcp[:, 3, 4:8], av3[1][:, :, D])

        def mul3(qb):
            dst = attn_pack[1][:, qb, D:2 * D]
            src_ = av3[qb // 4][:, qb % 4, 0:D]
            if qb % 2 == 0:
                nc.vector.tensor_scalar_mul(dst, src_, rcp[:, 3, qb:qb + 1])
            else:
                nc.scalar.activation(out=dst, in_=src_, func=Copy,
                                     scale=rcp[:, 3, qb:qb + 1])

        def tp1(qb):
            tp = ps_sq.tile([P, P], BF16, name=f"tp{qb}", tag="sq")
            nc.tensor.transpose(tp, attn_pack[1][:, qb, :], identity)
            if qb % 2 == 0:
                nc.vector.tensor_copy(attn_T[1][:, qb, :], tp)
            else:
                nc.scalar.copy(attn_T[1][:, qb, :], tp)

        # pair1 projection at quarter-column granularity: 32 independent
        # [128, 256] psum units spread across all three free pools (6 slots
        # in flight), single-engine fin per quarter (round-robin), one DMA
        # per m-block.
        qslots = [(ps_av, "av"), (ps_sc, "sc"), (ps_sq, "sq")]

        def proj1(m):
            fin = finp.tile([P, C], BF16, name=f"fin1_{m}", tag="fin")
            for qo in range(4):
                k = 4 * m + qo
                pool, tag = qslots[k % 3]
                ps = pool.tile([P, 256], F32, name=f"pj1_{m}_{qo}", tag=tag)
                mm(ps, attn_T[1][:, m, :],
                   wp_sb[1][:, qo * 256:(qo + 1) * 256],
                   start=True, stop=True, skip_group_check=True)
                dst = fin[:, qo * 256:(qo + 1) * 256]
                if k % 2 == 0:
                    nc.scalar.copy(dst, ps)
                else:
                    nc.vector.tensor_copy(dst, ps)
            nc.sync.dma_start(out=outB[m * P:(m + 1) * P, :], in_=fin)

        mul3(0)
        tp1(0)
        mul3(1)
        tp1(1)
        for qb in range(2, 8):
            mul3(qb)
            tp1(qb)
            proj1(qb - 2)
        proj1(6)
        proj1(7)


def _get_nc():
    if "nc" not in _CACHE:
        _CACHE["nc"] = _build()
    return _CACHE["nc"]


def kernel(x, wq, wk, wv, w_proj, b_proj):
    x = np.asarray(x, dtype=np.float32)
    wq = np.asarray(wq, dtype=np.float32)
    wk = np.asarray(wk, dtype=np.float32)
    wv = np.asarray(wv, dtype=np.float32)
    w_proj = np.asarray(w_proj, dtype=np.float32)
    b_proj = np.asarray(b_proj, dtype=np.float32)

    nc = _get_nc()
    in_maps = []
    for core in range(8):
        b, g = divmod(core, 4)
        s0 = g * DH
        p0 = slice(s0, s0 + P)            # pair0 rows (heads 4g, 4g+1)
        p1 = slice(s0 + P, s0 + 2 * P)    # pair1 rows
        sl = slice(s0, s0 + DH)
        in_maps.append({
            "xT": np.ascontiguousarray(x[b].T).astype(_BF),
            "wqk0": np.ascontiguousarray(
                np.hstack([wq[p0, :].T, wk[p0, :].T])).astype(_BF),
            "wqk1": np.ascontiguousarray(
                np.hstack([wq[p1, :].T, wk[p1, :].T])).astype(_BF),
            "wvT": np.ascontiguousarray(wv[sl, :].T).astype(_BF),
            "wpT": np.ascontiguousarray(w_proj[:, sl].T).astype(_BF),
        })

    res = run_bass_kernel_spmd(nc, in_maps, core_ids=list(range(8)),
                               trace=bool(int(os.environ.get("KERNEL_TRACE", "0"))))
    _CACHE["last_results"] = res
    acc = [np.zeros((NQ, C), np.float32) for _ in range(2)]
    for core in range(8):
        b = core // 4
        acc[b] += res.results[core]["outA"].astype(np.float32)
        acc[b] += res.results[core]["outB"].astype(np.float32)
    full = np.stack(acc)
    full += b_proj[None, None, :]
    return full.astype(np.float32)
